# revision 46
# baseline (speedup 1.0000x reference)
"""Trainium2 Bass kernel for a decoder layer (GQA attention + top-8/64 MoE).

Sharding over 8 NeuronCores:
  - Attention: (batch x kv-head-group) 8-way; each core computes 8 q-heads for
    one batch and produces a partial o_proj output (summed on host).
  - MoE: expert-parallel, 8 experts per core; routing/top-k + token dispatch on
    host between the two launches; experts are load-balanced across cores.

Precision plan: projections contract fp32r inputs into fp32 PSUM; the
score/softmax/context stage runs bf16 (q-hat/k-hat/v/es), o_proj weights bf16;
MoE runs fp8e4 (weights pre-scaled x512 host-side, activations x16, hidden x8)
with fp32 PSUM accumulation throughout.
"""

import os
import numpy as np
import ml_dtypes

B, S, D = 2, 1024, 2048
H, HKV, HD = 32, 4, 128
E, TOPK, MI = 64, 8, 768
EPS = 1e-6
T = B * S
P = 128
KT = D // P            # 16 k-chunks over D
NT = S // P            # 8 token tiles per batch
NH = H // HKV          # 8 q-heads per core
CHUNKS = [(0, 512), (512, 512)]
GM = MI // P           # 6 m-tiles over MI=768
BF16 = ml_dtypes.bfloat16
F8E4 = ml_dtypes.float8_e4m3
W_SCALE = 512.0        # fp8 scale for MoE weights (absmax ~0.11 -> 55)
A_SCALE = 16.0         # fp8 scale for MoE input acts (absmax ~5.6 -> 90)
H_SCALE = 8.0          # fp8 scale for MoE hidden (absmax ~11 -> 85)

# filled by kernel() when BASS_KERNEL_TRACE=1: [launch1_ns, launch2_ns]
LAST_EXEC_NS = []
LAST_RESULTS = []

_ACT_PATCHED = False


def _patch_act_tables():
    """Make Exp and Ln resolve to the single joint act-func table set so the
    attention inner loop never reloads the activation table (each reload is
    ~1.5us of Scalar-engine time and stalls the softmax chain).  Pure
    re-selection among real hardware tables - numerics are unchanged.  Fails
    soft: on any mismatch with the installed concourse, keep defaults."""
    global _ACT_PATCHED
    if _ACT_PATCHED:
        return
    _ACT_PATCHED = True
    try:
        import concourse.bacc as bacc_mod
        from concourse.hw_specs import get_activation_tables as _real

        def patched(arch):
            tabs = dict(_real(arch))
            joint = "natural_log_exp_and_others"
            if joint not in tabs:
                return tabs
            exp_ln = {f for f in tabs[joint]
                      if getattr(f, "name", "").lower() in ("exp", "ln")}
            if len(exp_ln) != 2:
                return tabs
            return {name: (fns if name == joint
                           else {f for f in fns if f not in exp_ln})
                    for name, fns in tabs.items()}

        bacc_mod.get_activation_tables = patched
    except Exception:
        pass


def _build_attn():
    import concourse.tile as tile
    from concourse import bacc, mybir

    F32 = mybir.dt.float32
    F32R = mybir.dt.float32r
    BF = mybir.dt.bfloat16
    AF = mybir.ActivationFunctionType

    nc = bacc.Bacc("TRN2", target_bir_lowering=False, debug=False, num_devices=8)
    xnt = nc.dram_tensor("xnt", [D, S], BF, kind="ExternalInput").ap()
    qwt = nc.dram_tensor("qwt", [D, NH * HD], BF, kind="ExternalInput").ap()
    kwt = nc.dram_tensor("kwt", [D, HD], BF, kind="ExternalInput").ap()
    vwt = nc.dram_tensor("vwt", [D, HD], BF, kind="ExternalInput").ap()
    owt = nc.dram_tensor("owt", [NH * HD, D], BF, kind="ExternalInput").ap()
    qn = nc.dram_tensor("qn", [P, 1], F32, kind="ExternalInput").ap()
    kn = nc.dram_tensor("kn", [P, 1], F32, kind="ExternalInput").ap()
    ones_kb = nc.dram_tensor("ones_kb", [P, 1], BF, kind="ExternalInput").ap()
    tri = nc.dram_tensor("tri", [P, P], BF, kind="ExternalInput").ap()
    ident = nc.dram_tensor("ident", [P, P], F32, kind="ExternalInput").ap()
    part = nc.dram_tensor("part", [D, S], F32, kind="ExternalOutput").ap()

    XC = 8                # xnt arrives in XC chunks of KT//XC k-tiles each
    KC = KT // XC
    xnt_r = xnt.rearrange("(o p) t -> p o t", p=P)
    qwt_r = qwt.rearrange("(o p) m -> p o m", p=P)
    kwt_r = kwt.rearrange("(o p) m -> p o m", p=P)
    vwt_r = vwt.rearrange("(o p) m -> p o m", p=P)
    owt_r = owt.rearrange("(o p) d -> p o d", p=P)
    part_r = part.rearrange("(o p) t -> p o t", p=P)

    with tile.TileContext(nc) as tc:
        with (
            tc.tile_pool(name="cst", bufs=1) as cst,
            tc.tile_pool(name="big", bufs=1) as big,
            tc.tile_pool(name="wstr", bufs=2) as wstr,
            tc.tile_pool(name="work", bufs=2) as work,
            tc.tile_pool(name="rows", bufs=2) as rows,
            tc.tile_pool(name="accp", bufs=3, space="PSUM") as accp,
            tc.tile_pool(name="scp", bufs=3, space="PSUM") as scp,
            tc.tile_pool(name="rowp", bufs=2, space="PSUM") as rowp,
        ):
            ones_kb_s = cst.tile([P, 1], BF)
            tri_s = cst.tile([P, P], BF)
            ident_s = cst.tile([P, P], F32)
            qn_s = cst.tile([P, 1], F32)
            kn_s = cst.tile([P, 1], F32)
            eps_s = cst.tile([P, 1], F32)
            nc.vector.memset(eps_s[:], float(EPS * HD))
            epsp_s = cst.tile([P, 1], F32)
            nc.vector.memset(epsp_s[:], float(EPS))
            nc.sync.dma_start(ones_kb_s[:], ones_kb)
            nc.sync.dma_start(tri_s[:], tri)
            nc.sync.dma_start(ident_s[:], ident)
            nc.sync.dma_start(qn_s[:], qn)
            nc.sync.dma_start(kn_s[:], kn)

            kwt_s = wstr.tile([P, KT, HD], BF, tag="qwh")
            vwt_s = wstr.tile([P, KT, HD], BF, tag="qwh")
            nc.sync.dma_start(kwt_s[:], kwt_r)
            nc.scalar.dma_start(vwt_s[:], vwt_r)
            # input arrives in XC chunks, alternating DGE queues so both DMA
            # paths pull the 4MB load in parallel
            xcs = []
            for c in range(XC):
                xc = big.tile([P, KC, S], BF, name=f"xc{c}")
                eng = nc.sync if c % 2 == 0 else nc.scalar
                eng.dma_start(xc[:], xnt_r[:, c * KC:(c + 1) * KC, :])
                xcs.append(xc)

            def xk(k):
                return xcs[k // KC][:, k % KC, :]

            # ---- K and V (both chunks interleaved so every arriving x-chunk
            # immediately feeds 4 matmuls) ----
            kht = big.tile([P, S], BF)          # k*kn_w, feature-major [hd, t']
            rk = big.tile([P, NT], F32)         # per-token 1/sqrt(sumsq+eps*HD), col i
            vtm = big.tile([P, NT, P], BF)      # v token-major tiles [t', hd]
            psk = [accp.tile([P, 512], F32, tag="acc", name=f"psk{ci}")
                   for ci in range(2)]
            psv = [scp.tile([P, 512], F32, tag="sc", name=f"psv{ci}")
                   for ci in range(2)]
            for k in range(KT):
                for ci, (c0, cw) in enumerate(CHUNKS):
                    nc.tensor.matmul(psk[ci][:, :cw], kwt_s[:, k, :],
                                     xk(k)[:, c0:c0 + cw],
                                     start=(k == 0), stop=(k == KT - 1))
                    nc.tensor.matmul(psv[ci][:, :cw], vwt_s[:, k, :],
                                     xk(k)[:, c0:c0 + cw],
                                     start=(k == 0), stop=(k == KT - 1))
            for ci, (c0, cw) in enumerate(CHUNKS):
                kraw = work.tile([P, 512], F32, tag="kraw", name=f"kraw{ci}")
                nc.scalar.copy(kraw[:, :cw], psk[ci][:, :cw])
                nc.vector.tensor_scalar_mul(kht[:, c0:c0 + cw], psk[ci][:, :cw], kn_s[:])
                vraw = work.tile([P, 512], F32, tag="vraw", name=f"vraw{ci}")
                nc.vector.tensor_copy(vraw[:, :cw], psv[ci][:, :cw])
                for j in range(cw // P):
                    i = (c0 + j * P) // P
                    ptr = scp.tile([P, 512], F32, tag="sc", name=f"ptrk{ci}")
                    nc.tensor.transpose(ptr[:, :P], kraw[:, j * P:(j + 1) * P], ident_s[:])
                    ksq = work.tile([P, P], F32, tag="ksq")
                    nc.scalar.square(ksq[:], ptr[:, :P])
                    ksum = work.tile([P, 1], F32, tag="ksum")
                    nc.vector.tensor_reduce(ksum[:], ksq[:], mybir.AxisListType.X,
                                            mybir.AluOpType.add)
                    klog = work.tile([P, 1], F32, tag="kst")
                    nc.scalar.activation(klog[:], ksum[:], AF.Ln, bias=epsp_s[:],
                                         scale=float(1.0 / HD))
                    nc.scalar.activation(rk[:, i:i + 1], klog[:], AF.Exp,
                                         scale=-0.5)
                    ptv = scp.tile([P, 512], F32, tag="sc", name=f"ptrv{ci}")
                    nc.tensor.transpose(ptv[:, :P], vraw[:, j * P:(j + 1) * P], ident_s[:])
                    nc.vector.tensor_copy(vtm[:, i, :], ptv[:, :P])

            # ---- heads (software-pipelined: C1(h+1) stages overlap C2(h)) ----
            ctx = big.tile([P, NH, S], BF)
            qhat_t = {}
            st_qraw = {}
            st_rrec = {}

            def c1a(h):
                # q projection + squared sums; prow matmuls last so the DVE
                # square chain is covered by the second chunk's projection
                qw_h = wstr.tile([P, KT, P], BF, tag="qwh", name=f"qw{h}")
                nc.scalar.dma_start(qw_h[:], qwt_r[:, :, h * P:(h + 1) * P])
                qhat_t[h] = work.tile([P, S], BF, tag="qhat", name=f"qhat{h}")
                q2s = []
                for ci, (c0, cw) in enumerate(CHUNKS):
                    psq = accp.tile([P, 512], F32, tag="acc", name=f"psq{h}")
                    for k in range(KT):
                        nc.tensor.matmul(psq[:, :cw], qw_h[:, k, :], xk(k)[:, c0:c0 + cw],
                                         start=(k == 0), stop=(k == KT - 1))
                    qraw = work.tile([P, 512], BF, tag=f"qraw{ci}", name=f"qraw{h}")
                    nc.vector.tensor_copy(qraw[:, :cw], psq[:, :cw])
                    q2 = work.tile([P, 512], BF, tag="q2", name=f"q2{h}")
                    nc.vector.tensor_tensor(q2[:, :cw], qraw[:, :cw], qraw[:, :cw],
                                            mybir.AluOpType.mult)
                    st_qraw[(h, ci)] = qraw
                    q2s.append(q2)
                for ci, (c0, cw) in enumerate(CHUNKS):
                    prow = rowp.tile([1, 512], F32, tag="row", name=f"prow{h}")
                    nc.tensor.matmul(prow[:, :cw], ones_kb_s[:], q2s[ci][:, :cw],
                                     start=True, stop=True)
                    st_rrec[(h, ci)] = prow

            def c1b(h):
                # rsqrt rows (Ln+Exp keeps the single act table resident); the
                # GPSIMD broadcast starts here so it's long done by c1c
                for ci, (c0, cw) in enumerate(CHUNKS):
                    prow = st_rrec[(h, ci)]
                    rrow = rows.tile([1, 512], F32, tag="rowa", name=f"rrow{h}")
                    nc.scalar.activation(rrow[:, :cw], prow[:, :cw], AF.Ln,
                                         bias=eps_s[:1, :])
                    rrec = rows.tile([1, 512], F32R, tag="rowb", name=f"rrec{h}")
                    nc.scalar.activation(rrec[:, :cw], rrow[:, :cw], AF.Exp,
                                         scale=-0.5)
                    rrb = work.tile([P, 512], F32R, tag="rrb", name=f"rrb{h}")
                    nc.gpsimd.partition_broadcast(rrb[:, :cw], rrec[:1, :cw])
                    st_rrec[(h, ci)] = rrb

            def c1c(h):
                qhat = qhat_t[h]
                for ci, (c0, cw) in enumerate(CHUNKS):
                    rrb = st_rrec.pop((h, ci))
                    qraw = st_qraw.pop((h, ci))
                    nc.vector.scalar_tensor_tensor(qhat[:, c0:c0 + cw], qraw[:, :cw],
                                                   qn_s[:], rrb[:, :cw],
                                                   mybir.AluOpType.mult,
                                                   mybir.AluOpType.mult)

            def c2(h, mid_hooks=()):
                qhat = qhat_t.pop(h)
                for ci, (c0, cw) in enumerate(CHUNKS):
                    nvalid = 4 if ci == 0 else 8
                    pctx = accp.tile([P, 512], F32, tag="acc", name=f"pctx{h}")
                    pden = rowp.tile([1, 512], F32, tag="row", name=f"pden{h}")
                    prev = None
                    for ii in range(nvalid):
                        pss = scp.tile([P, 512], F32, tag="sc", name=f"pss{h}")
                        nc.tensor.matmul(pss[:, :cw], kht[:, ii * P:(ii + 1) * P],
                                         qhat[:, c0:c0 + cw], start=True, stop=True)
                        es = work.tile([P, 512], BF, tag="es", bufs=3, name=f"es{h}")
                        mp = ii - 4 * ci
                        lo = mp * P if mp >= 0 else 0
                        # exp only the causally-valid columns; zero the rest
                        nc.scalar.activation(es[:, lo:cw], pss[:, lo:cw], AF.Exp,
                                             scale=rk[:, ii:ii + 1])
                        if mp >= 0:
                            if lo > 0:
                                nc.vector.memset(es[:, :lo], 0.0)
                            nc.vector.tensor_tensor(es[:, lo:lo + P], es[:, lo:lo + P],
                                                    tri_s[:],
                                                    mybir.AluOpType.mult)
                        if ii == 0 and ci < len(mid_hooks):
                            mid_hooks[ci]()
                        if prev is not None:
                            pi, pes = prev
                            nc.tensor.matmul(pctx[:, :cw], vtm[:, pi, :], pes[:, :cw],
                                             start=(pi == 0), stop=False)
                            nc.tensor.matmul(pden[:, :cw], ones_kb_s[:], pes[:, :cw],
                                             start=(pi == 0), stop=False)
                        prev = (ii, es)
                    pi, pes = prev
                    nc.tensor.matmul(pctx[:, :cw], vtm[:, pi, :], pes[:, :cw],
                                     start=(pi == 0), stop=True)
                    nc.tensor.matmul(pden[:, :cw], ones_kb_s[:], pes[:, :cw],
                                     start=(pi == 0), stop=True)
                    # express-release pctx into SBUF; the normalize chain then
                    # runs entirely on GPSIMD so it never blocks the DVE FIFO
                    # that feeds the next chunk's score masks
                    ctxr = work.tile([P, 512], BF, tag="ctxr", name=f"ctxr{h}")
                    nc.vector.tensor_copy(ctxr[:, :cw], pctx[:, :cw])
                    rden = rows.tile([1, 512], F32, tag="rowd", name=f"rden{h}")
                    nc.vector.reciprocal_approx_fast(rden[:, :cw], pden[:, :cw])
                    rdb = work.tile([P, 512], F32, tag="rdb", name=f"rdb{h}")
                    nc.gpsimd.partition_broadcast(rdb[:, :cw], rden[:1, :cw])
                    nc.gpsimd.tensor_tensor(ctx[:, h, c0:c0 + cw], ctxr[:, :cw],
                                            rdb[:, :cw], mybir.AluOpType.mult)

            c1a(0)
            c1b(0)
            c1c(0)
            for h in range(NH):
                if h + 1 < NH:
                    c1a(h + 1)
                    c2(h, mid_hooks=(lambda: c1b(h + 1), lambda: c1c(h + 1)))
                else:
                    c2(h)

            # ---- o_proj (partial); stores alternate across both DGE queues ----
            for md in range(KT):
                ow_md = wstr.tile([P, NH, P], BF, tag="owmd")
                nc.sync.dma_start(ow_md[:], owt_r[:, :, md * P:(md + 1) * P])
                for ci, (c0, cw) in enumerate(CHUNKS):
                    pso = accp.tile([P, 512], F32, tag="acc")
                    for h2 in range(NH):
                        nc.tensor.matmul(pso[:, :cw], ow_md[:, h2, :], ctx[:, h2, c0:c0 + cw],
                                         start=(h2 == 0), stop=(h2 == NH - 1))
                    osb = work.tile([P, 512], F32, tag="osb")
                    nc.vector.tensor_copy(osb[:, :cw], pso[:, :cw])
                    eng = nc.scalar if (md * 2 + ci) % 2 == 0 else nc.sync
                    eng.dma_start(part_r[:, md, c0:c0 + cw], osb[:, :cw])

    nc.compile()
    return nc


def _build_moe(segs, CT):
    """segs: list of (offset, cap) per slot (same layout on all cores)."""
    import concourse.tile as tile
    from concourse import bacc, mybir

    F32 = mybir.dt.float32
    BF = mybir.dt.bfloat16
    F8 = mybir.dt.float8e4
    AF = mybir.ActivationFunctionType
    NS = len(segs)
    GU_DESCALE = 1.0 / (W_SCALE * A_SCALE)          # psg -> real gate/up values
    H_FOLD = H_SCALE / (W_SCALE * A_SCALE)          # psu * sg -> hidden * H_SCALE
    DN_DESCALE = 1.0 / (W_SCALE * H_SCALE)          # psd -> real down output

    nc = bacc.Bacc("TRN2", target_bir_lowering=False, debug=False, num_devices=8)
    xgt = nc.dram_tensor("xgt", [D, CT], F8, kind="ExternalInput").ap()
    gwt = nc.dram_tensor("gwt", [NS, D, MI], F8, kind="ExternalInput").ap()
    uwt = nc.dram_tensor("uwt", [NS, D, MI], F8, kind="ExternalInput").ap()
    dnt = nc.dram_tensor("dnt", [NS, MI, D], F8, kind="ExternalInput").ap()
    mout = nc.dram_tensor("mout", [D, CT], BF, kind="ExternalOutput").ap()

    xgt_r = xgt.rearrange("(o p) c -> p o c", p=P)
    mout_r = mout.rearrange("(o p) c -> p o c", p=P)

    with tile.TileContext(nc) as tc:
        with (
            tc.tile_pool(name="xp", bufs=2) as xp,
            tc.tile_pool(name="dnp", bufs=2) as dnp,
            tc.tile_pool(name="wp", bufs=8) as wp,
            tc.tile_pool(name="hp", bufs=2) as hp,
            tc.tile_pool(name="op", bufs=3) as op_,
            tc.tile_pool(name="gps", bufs=1, space="PSUM") as gps,
            tc.tile_pool(name="dps", bufs=2, space="PSUM") as dps,
        ):
            DR = mybir.MatmulPerfMode.DoubleRow

            def emit_down_md(pend, md):
                # one down-proj output tile of the PREVIOUS slot
                dn_p, h_p, off_p, cs_p = pend
                psd = dps.tile([P, 512], F32, tag="d", name=f"psd{md}")
                for jp in range(GM // 2):
                    nc.tensor.matmul(psd[:, :cs_p],
                                     dn_p[:, 2 * jp:2 * jp + 2, md * P:(md + 1) * P],
                                     h_p[:, 2 * jp:2 * jp + 2, :],
                                     start=(jp == 0), stop=(jp == GM // 2 - 1),
                                     perf_mode=DR)
                ob = op_.tile([P, 512], BF, tag="ob", name=f"ob{md}")
                nc.vector.tensor_scalar_mul(ob[:, :cs_p], psd[:, :cs_p],
                                            float(DN_DESCALE))
                nc.scalar.dma_start(mout_r[:, md, off_p:off_p + cs_p], ob[:, :cs_p])

            # warm the Silu act table while the first DMAs are in flight
            warm = hp.tile([P, 1], F32, tag="warm", bufs=1)
            nc.vector.memset(warm[:], 0.0)
            nc.scalar.activation(warm[:], warm[:], AF.Silu)

            # prefetch each slot's tokens one slot ahead on the weight queue
            xg_tiles = {}

            def fetch_xg(s):
                off, cs = segs[s]
                xg_tiles[s] = xp.tile([P, KT, cs], F8, tag="xg", name=f"xg{s}")
                nc.sync.dma_start(xg_tiles[s][:], xgt_r[:, :, off:off + cs])

            fetch_xg(0)
            pend = None
            for s, (off, cs) in enumerate(segs):
                xg_s = xg_tiles.pop(s)

                gw_r = gwt[s].rearrange("(o p) i -> p o i", p=P)
                uw_r = uwt[s].rearrange("(o p) i -> p o i", p=P)

                # gate pass, with the previous slot's down-proj interleaved so
                # the PE never drains at slot boundaries (PSUM: 6 gate + 2 down)
                psg = [gps.tile([P, 512], F32, tag=f"g{m}", name=f"psg{m}") for m in range(GM)]
                uw_t = []
                KP = KT // 2
                for kp in range(KP):
                    gw_k = wp.tile([P, 2, MI], F8, tag="gw")
                    nc.sync.dma_start(gw_k[:], gw_r[:, 2 * kp:2 * kp + 2, :])
                    uw_k = wp.tile([P, 2, MI], F8, tag="uw", bufs=KP, name=f"uw{kp}")
                    nc.sync.dma_start(uw_k[:], uw_r[:, 2 * kp:2 * kp + 2, :])
                    uw_t.append(uw_k)
                    for m in range(GM):
                        nc.tensor.matmul(psg[m][:, :cs],
                                         gw_k[:, :, m * P:(m + 1) * P],
                                         xg_s[:, 2 * kp:2 * kp + 2, :],
                                         start=(kp == 0), stop=(kp == KP - 1),
                                         perf_mode=DR)
                    if pend is not None:
                        emit_down_md(pend, 2 * kp)
                        emit_down_md(pend, 2 * kp + 1)
                if s + 1 < len(segs):
                    fetch_xg(s + 1)
                dn_s = dnp.tile([P, GM, D], F8, tag="dn")
                nc.sync.dma_start(dn_s[:], dnt[s].rearrange("(o p) d -> p o d", p=P))
                sg = hp.tile([P, GM, cs], F32, tag="sg")
                for m in range(GM):
                    nc.scalar.activation(sg[:, m], psg[m][:, :cs], AF.Silu,
                                         scale=float(GU_DESCALE))

                psu = [gps.tile([P, 512], F32, tag=f"g{m}", name=f"psu{m}") for m in range(GM)]
                for kp in range(KP):
                    for m in range(GM):
                        nc.tensor.matmul(psu[m][:, :cs],
                                         uw_t[kp][:, :, m * P:(m + 1) * P],
                                         xg_s[:, 2 * kp:2 * kp + 2, :],
                                         start=(kp == 0), stop=(kp == KP - 1),
                                         perf_mode=DR)
                hvals = hp.tile([P, GM, cs], F8, tag="h")
                for m in range(GM):
                    nc.vector.scalar_tensor_tensor(hvals[:, m], psu[m][:, :cs],
                                                   float(H_FOLD), sg[:, m],
                                                   mybir.AluOpType.mult,
                                                   mybir.AluOpType.mult)
                pend = (dn_s, hvals, off, cs)

            for md in range(KT):
                emit_down_md(pend, md)

    nc.compile()
    return nc


def _refine_logits(tokens, x, in_ln_w, q_w, k_w, v_w, o_w, qn_w, kn_w,
                   post_ln_w, router_w):
    """Exact (fp64) router logits for the given global token ids."""
    out = {}
    nrep = H // HKV
    x64 = x.astype(np.float64)
    qw64 = q_w.astype(np.float64)
    kw64 = k_w.astype(np.float64)
    vw64 = v_w.astype(np.float64)
    ow64 = o_w.astype(np.float64)
    rw64 = router_w.astype(np.float64)
    for b in sorted({int(t) // S for t in tokens}):
        xb = x64[b]
        xn = xb / np.sqrt((xb ** 2).mean(-1, keepdims=True) + EPS) * in_ln_w
        k = (xn @ kw64.T).reshape(S, HKV, HD)
        k = k / np.sqrt((k ** 2).mean(-1, keepdims=True) + EPS) * kn_w
        v = (xn @ vw64.T).reshape(S, HKV, HD)
        for t in [int(t) for t in tokens if int(t) // S == b]:
            p = t % S
            q = (xn[p] @ qw64.T).reshape(H, HD)
            q = q / np.sqrt((q ** 2).mean(-1, keepdims=True) + EPS) * qn_w
            ctx = np.empty((H, HD))
            for h in range(H):
                g = h // nrep
                sc = (k[:p + 1, g] @ q[h]) * (HD ** -0.5)
                eo = np.exp(sc - sc.max())
                ctx[h] = (eo / eo.sum()) @ v[:p + 1, g]
            at = ctx.reshape(-1) @ ow64.T
            h1t = xb[p] + at
            xmt = h1t / np.sqrt((h1t ** 2).mean() + EPS) * post_ln_w
            out[t] = xmt @ rw64.T
    return out


def _run(nc, in_maps, trace):
    from concourse.bass_utils import run_bass_kernel_spmd
    res = run_bass_kernel_spmd(nc, in_maps, core_ids=list(range(8)), trace=trace)
    if trace:
        LAST_EXEC_NS.append(res.exec_time_ns)
        LAST_RESULTS.append(res)
    return res.results


def kernel(x, in_ln_w, q_w, k_w, v_w, o_w, qn_w, kn_w, post_ln_w,
           router_w, gate_up_w, down_w):
    trace = os.environ.get("BASS_KERNEL_TRACE", "0") == "1"
    LAST_EXEC_NS.clear()
    LAST_RESULTS.clear()
    _patch_act_tables()

    x = np.asarray(x, np.float32)
    in_ln_w = np.asarray(in_ln_w, np.float32)
    q_w = np.asarray(q_w, np.float32)
    k_w = np.asarray(k_w, np.float32)
    v_w = np.asarray(v_w, np.float32)
    o_w = np.asarray(o_w, np.float32)
    qn_w = np.asarray(qn_w, np.float32)
    kn_w = np.asarray(kn_w, np.float32)
    post_ln_w = np.asarray(post_ln_w, np.float32)
    router_w = np.asarray(router_w, np.float32)
    gate_up_w = np.asarray(gate_up_w, np.float32)
    down_w = np.asarray(down_w, np.float32)

    # ---------- host prep: pre-normed input, transposed weight shards ----------
    xT = [np.ascontiguousarray(x[b].T) for b in range(B)]          # [D, S]
    rms = 1.0 / np.sqrt((x.astype(np.float64) ** 2).mean(-1) + EPS)  # [B, S]
    xntT = [np.ascontiguousarray(in_ln_w[:, None] * xT[b] * rms[b][None, :].astype(np.float32)).astype(BF16)
            for b in range(B)]

    tri = np.triu(np.ones((P, P), np.float32)).astype(BF16)  # [t', t] valid t>=t'
    ident = np.eye(P, dtype=np.float32)
    ones_kb = np.ones((P, 1), np.float32).astype(BF16)
    qn_col = np.ascontiguousarray(qn_w.reshape(P, 1))
    kn_col = np.ascontiguousarray(kn_w.reshape(P, 1))

    attn_nc = _build_attn()
    in_maps1 = []
    for c in range(8):
        b, g = c // HKV, c % HKV
        qslice = np.ascontiguousarray(q_w[g * NH * HD:(g + 1) * NH * HD].T).astype(BF16)
        kslice = np.ascontiguousarray(k_w[g * HD:(g + 1) * HD].T).astype(BF16)
        vslice = np.ascontiguousarray(v_w[g * HD:(g + 1) * HD].T).astype(BF16)
        oslice = np.ascontiguousarray(o_w[:, g * NH * HD:(g + 1) * NH * HD].T).astype(BF16)
        in_maps1.append({
            "xnt": xntT[b], "qwt": qslice, "kwt": kslice, "vwt": vslice,
            "owt": oslice, "qn": qn_col, "kn": kn_col,
            "ones_kb": ones_kb,
            "tri": tri, "ident": ident,
        })
    res1 = _run(attn_nc, in_maps1, trace)

    # ---------- residual + post-norm + routing (host) ----------
    attnT = [res1[4 * b + 0]["part"] + res1[4 * b + 1]["part"]
             + res1[4 * b + 2]["part"] + res1[4 * b + 3]["part"] for b in range(B)]
    if os.environ.get("BASS_KERNEL_DEBUG", "0") == "1":
        np.save("/root/problem/dbg_attnT.npy", np.stack(attnT))
        np.save("/root/problem/dbg_parts.npy",
                np.stack([res1[c]["part"] for c in range(8)]))
    h1T = np.concatenate([xT[b] + attnT[b] for b in range(B)], axis=1)  # [D, T]
    mrms = 1.0 / np.sqrt((h1T.astype(np.float64) ** 2).mean(0) + EPS)   # [T]
    xmT = (post_ln_w[:, None] * h1T * mrms[None, :].astype(np.float32)).astype(np.float32)

    logits = (xmT.T @ router_w.T).astype(np.float32)                    # [T, E]
    lmax = logits.max(-1, keepdims=True)
    ex = np.exp(logits - lmax)
    probs = ex / ex.sum(-1, keepdims=True)
    order = np.argsort(-probs, axis=-1, kind="stable")
    idx = order[:, :TOPK]                                               # [T, 8]
    vals = np.take_along_axis(probs, idx, axis=-1)
    vals = vals / vals.sum(-1, keepdims=True)

    # Top-8 selections whose prob gap is within our attention error bound are
    # ambiguous: recompute those tokens' logits exactly (fp64) on host so the
    # expert choice matches the fp32 reference.
    srt = np.sort(probs, axis=-1)[:, ::-1]
    amb = np.where(srt[:, TOPK - 1] - srt[:, TOPK] < 3e-4)[0]
    if os.environ.get("BASS_KERNEL_DEBUG", "0") == "1":
        print(f"[kernel] ambiguous tokens: {len(amb)}")
    if len(amb):
        refined = _refine_logits(amb, x, in_ln_w, q_w, k_w, v_w, o_w,
                                 qn_w, kn_w, post_ln_w, router_w)
        for t, lg in refined.items():
            eo = np.exp(lg - lg.max())
            pb = eo / eo.sum()
            o8 = np.argsort(-pb, kind="stable")[:TOPK]
            idx[t] = o8
            v8 = pb[o8]
            vals[t] = (v8 / v8.sum()).astype(np.float32)

    # token lists per expert
    tok_ids = [None] * E
    tok_w = [None] * E
    flat_tok = np.repeat(np.arange(T), TOPK)
    flat_e = idx.ravel()
    flat_w = vals.ravel()
    ords = np.argsort(flat_e, kind="stable")
    bounds = np.searchsorted(flat_e[ords], np.arange(E + 1))
    for e in range(E):
        sel = ords[bounds[e]:bounds[e + 1]]
        tok_ids[e] = flat_tok[sel]
        tok_w[e] = flat_w[sel].astype(np.float32)
    counts = np.array([len(t) for t in tok_ids])

    # balanced assignment: rank-grouped — slot s of core c gets expert ranked 8s+c
    rank = np.argsort(-counts, kind="stable")
    assign = [[int(rank[8 * s + c]) for s in range(8)] for c in range(8)]
    caps = []
    for s in range(8):
        cap = int(max(counts[rank[8 * s + c]] for c in range(8)))
        # multiple of 16 so fp8 DoubleRow APs meet the 16B step alignment
        caps.append(max(16, (cap + 15) // 16 * 16))
    offs = np.concatenate([[0], np.cumsum(caps)]).astype(int)
    CT = int(offs[-1])
    segs = [(int(offs[s]), int(caps[s])) for s in range(8)]

    xm_f8 = np.clip(xmT * A_SCALE, -240.0, 240.0).astype(F8E4)
    moe_nc = _build_moe(segs, CT)
    in_maps2 = []
    for c in range(8):
        xg = np.zeros((D, CT), F8E4)
        gw = np.empty((8, D, MI), F8E4)
        uw = np.empty((8, D, MI), F8E4)
        dn = np.empty((8, MI, D), F8E4)
        for s in range(8):
            e = assign[c][s]
            ids = tok_ids[e]
            xg[:, offs[s]:offs[s] + len(ids)] = xm_f8[:, ids]
            gw[s] = np.clip(gate_up_w[e, :MI].T * W_SCALE, -240, 240).astype(F8E4)
            uw[s] = np.clip(gate_up_w[e, MI:].T * W_SCALE, -240, 240).astype(F8E4)
            dn[s] = np.clip(down_w[e].T * W_SCALE, -240, 240).astype(F8E4)
        in_maps2.append({"xgt": xg, "gwt": gw, "uwt": uw, "dnt": dn})
    res2 = _run(moe_nc, in_maps2, trace)

    # ---------- scatter-add + final residual (host) ----------
    moT = np.zeros((D, T), np.float32)
    for c in range(8):
        mo = res2[c]["mout"].astype(np.float32)
        for s in range(8):
            e = assign[c][s]
            ids = tok_ids[e]
            if len(ids):
                moT[:, ids] += tok_w[e][None, :] * mo[:, offs[s]:offs[s] + len(ids)]

    if os.environ.get("BASS_KERNEL_DEBUG", "0") == "1":
        np.save("/root/problem/dbg_xmT.npy", xmT)
        np.save("/root/problem/dbg_idx.npy", idx)
        np.save("/root/problem/dbg_vals.npy", vals)
        np.save("/root/problem/dbg_moT.npy", moT)

    outT = h1T + moT
    return np.ascontiguousarray(outT.T).reshape(B, S, D).astype(np.float32)


# revision 56
# speedup vs baseline: 1.2517x; 1.2517x over previous
"""Trainium2 Bass kernel for a decoder layer (GQA attention + top-8/64 MoE).

Sharding over 8 NeuronCores:
  - Attention: (batch x kv-head-group) 8-way; each core computes 8 q-heads for
    one batch and produces a partial o_proj output (summed on host).
  - MoE: expert-parallel, 8 experts per core; routing/top-k + token dispatch on
    host between the two launches; experts are load-balanced across cores.

Precision plan: projections contract fp32r inputs into fp32 PSUM; the
score/softmax/context stage runs bf16 (q-hat/k-hat/v/es), o_proj weights bf16;
MoE runs fp8e4 (weights pre-scaled x512 host-side, activations x16, hidden x8)
with fp32 PSUM accumulation throughout.
"""

import os
import numpy as np
import ml_dtypes

B, S, D = 2, 1024, 2048
H, HKV, HD = 32, 4, 128
E, TOPK, MI = 64, 8, 768
EPS = 1e-6
T = B * S
P = 128
KT = D // P            # 16 k-chunks over D
NT = S // P            # 8 token tiles per batch
NH = H // HKV          # 8 q-heads per core
CHUNKS = [(0, 512), (512, 512)]
GM = MI // P           # 6 m-tiles over MI=768
BF16 = ml_dtypes.bfloat16
F8E4 = ml_dtypes.float8_e4m3
W_SCALE = 512.0        # fp8 scale for MoE weights (absmax ~0.11 -> 55)
A_SCALE = 16.0         # fp8 scale for MoE input acts (absmax ~5.6 -> 90)
H_SCALE = 8.0          # fp8 scale for MoE hidden (absmax ~11 -> 85)

# filled by kernel() when BASS_KERNEL_TRACE=1: [launch1_ns, launch2_ns]
LAST_EXEC_NS = []
LAST_RESULTS = []

_ACT_PATCHED = False


def _patch_act_tables():
    """Make Exp and Ln resolve to the single joint act-func table set so the
    attention inner loop never reloads the activation table (each reload is
    ~1.5us of Scalar-engine time and stalls the softmax chain).  Pure
    re-selection among real hardware tables - numerics are unchanged.  Fails
    soft: on any mismatch with the installed concourse, keep defaults."""
    global _ACT_PATCHED
    if _ACT_PATCHED:
        return
    _ACT_PATCHED = True
    try:
        import concourse.bacc as bacc_mod
        from concourse.hw_specs import get_activation_tables as _real

        def patched(arch):
            tabs = dict(_real(arch))
            joint = "natural_log_exp_and_others"
            if joint not in tabs:
                return tabs
            exp_ln = {f for f in tabs[joint]
                      if getattr(f, "name", "").lower() in ("exp", "ln")}
            if len(exp_ln) != 2:
                return tabs
            return {name: (fns if name == joint
                           else {f for f in fns if f not in exp_ln})
                    for name, fns in tabs.items()}

        bacc_mod.get_activation_tables = patched
    except Exception:
        pass


def _build_attn():
    import concourse.tile as tile
    from concourse import bacc, mybir

    F32 = mybir.dt.float32
    F32R = mybir.dt.float32r
    BF = mybir.dt.bfloat16
    AF = mybir.ActivationFunctionType

    nc = bacc.Bacc("TRN2", target_bir_lowering=False, debug=False, num_devices=8)
    xnt = nc.dram_tensor("xnt", [D, S], BF, kind="ExternalInput").ap()
    qwt = nc.dram_tensor("qwt", [D, NH * HD], BF, kind="ExternalInput").ap()
    kwt = nc.dram_tensor("kwt", [D, HD], BF, kind="ExternalInput").ap()
    vwt = nc.dram_tensor("vwt", [D, HD], BF, kind="ExternalInput").ap()
    owt = nc.dram_tensor("owt", [NH * HD, D], BF, kind="ExternalInput").ap()
    qn = nc.dram_tensor("qn", [P, 1], F32, kind="ExternalInput").ap()
    kn = nc.dram_tensor("kn", [P, 1], F32, kind="ExternalInput").ap()
    ones_kb = nc.dram_tensor("ones_kb", [P, 1], BF, kind="ExternalInput").ap()
    ones_m = nc.dram_tensor("ones_m", [1, P], F32R, kind="ExternalInput").ap()
    tri = nc.dram_tensor("tri", [P, P], BF, kind="ExternalInput").ap()
    ident = nc.dram_tensor("ident", [P, P], F32, kind="ExternalInput").ap()
    part = nc.dram_tensor("part", [D, S], F32, kind="ExternalOutput").ap()

    XC = 8                # xnt arrives in XC chunks of KT//XC k-tiles each
    KC = KT // XC
    xnt_r = xnt.rearrange("(o p) t -> p o t", p=P)
    qwt_r = qwt.rearrange("(o p) m -> p o m", p=P)
    kwt_r = kwt.rearrange("(o p) m -> p o m", p=P)
    vwt_r = vwt.rearrange("(o p) m -> p o m", p=P)
    owt_r = owt.rearrange("(o p) d -> p o d", p=P)
    part_r = part.rearrange("(o p) t -> p o t", p=P)

    with tile.TileContext(nc) as tc:
        with (
            tc.tile_pool(name="cst", bufs=1) as cst,
            tc.tile_pool(name="big", bufs=1) as big,
            tc.tile_pool(name="wstr", bufs=2) as wstr,
            tc.tile_pool(name="work", bufs=2) as work,
            tc.tile_pool(name="rows", bufs=2) as rows,
            tc.tile_pool(name="accp", bufs=3, space="PSUM") as accp,
            tc.tile_pool(name="scp", bufs=2, space="PSUM") as scp,
            tc.tile_pool(name="rowp", bufs=2, space="PSUM") as rowp,
            tc.tile_pool(name="bcp", bufs=1, space="PSUM") as bcp,
        ):
            ones_kb_s = cst.tile([P, 1], BF)
            ones_m_s = cst.tile([1, P], F32R)
            tri_s = cst.tile([P, P], BF)
            ident_s = cst.tile([P, P], F32)
            qn_s = cst.tile([P, 1], F32)
            kn_s = cst.tile([P, 1], F32)
            eps_s = cst.tile([P, 1], F32)
            nc.vector.memset(eps_s[:], float(EPS * HD))
            epsp_s = cst.tile([P, 1], F32)
            nc.vector.memset(epsp_s[:], float(EPS))
            nc.sync.dma_start(ones_kb_s[:], ones_kb)
            nc.sync.dma_start(ones_m_s[:], ones_m)
            nc.sync.dma_start(tri_s[:], tri)
            nc.sync.dma_start(ident_s[:], ident)
            nc.sync.dma_start(qn_s[:], qn)
            nc.sync.dma_start(kn_s[:], kn)

            kwt_s = wstr.tile([P, KT, HD], BF, tag="qwh")
            vwt_s = wstr.tile([P, KT, HD], BF, tag="qwh")
            nc.sync.dma_start(kwt_s[:], kwt_r)
            nc.scalar.dma_start(vwt_s[:], vwt_r)
            # input arrives in XC chunks, alternating DGE queues so both DMA
            # paths pull the 4MB load in parallel
            xcs = []
            for c in range(XC):
                xc = big.tile([P, KC, S], BF, name=f"xc{c}")
                eng = nc.sync if c % 2 == 0 else nc.scalar
                eng.dma_start(xc[:], xnt_r[:, c * KC:(c + 1) * KC, :])
                xcs.append(xc)

            def xk(k):
                return xcs[k // KC][:, k % KC, :]

            # ---- K and V (both chunks interleaved so every arriving x-chunk
            # immediately feeds 4 matmuls) ----
            kht = big.tile([P, S], BF)          # k*kn_w, feature-major [hd, t']
            rk = big.tile([P, NT], F32)         # per-token 1/sqrt(sumsq+eps*HD), col i
            vtm = big.tile([P, NT, P], BF)      # v token-major tiles [t', hd]
            psk = [accp.tile([P, 512], F32, tag="acc", name=f"psk{ci}")
                   for ci in range(2)]
            psv = [scp.tile([P, 512], F32, tag="sc", name=f"psv{ci}")
                   for ci in range(2)]
            for k in range(KT):
                for ci, (c0, cw) in enumerate(CHUNKS):
                    nc.tensor.matmul(psk[ci][:, :cw], kwt_s[:, k, :],
                                     xk(k)[:, c0:c0 + cw],
                                     start=(k == 0), stop=(k == KT - 1))
                    nc.tensor.matmul(psv[ci][:, :cw], vwt_s[:, k, :],
                                     xk(k)[:, c0:c0 + cw],
                                     start=(k == 0), stop=(k == KT - 1))
            for ci, (c0, cw) in enumerate(CHUNKS):
                kraw = work.tile([P, 512], F32, tag="kraw", name=f"kraw{ci}")
                nc.scalar.copy(kraw[:, :cw], psk[ci][:, :cw])
                nc.vector.tensor_scalar_mul(kht[:, c0:c0 + cw], psk[ci][:, :cw], kn_s[:])
                vraw = work.tile([P, 512], F32, tag="vraw", name=f"vraw{ci}")
                nc.vector.tensor_copy(vraw[:, :cw], psv[ci][:, :cw])
                for j in range(cw // P):
                    i = (c0 + j * P) // P
                    ptr = scp.tile([P, 512], F32, tag="sc", name=f"ptrk{ci}")
                    nc.tensor.transpose(ptr[:, :P], kraw[:, j * P:(j + 1) * P], ident_s[:])
                    ksq = work.tile([P, P], F32, tag="ksq")
                    nc.scalar.square(ksq[:], ptr[:, :P])
                    ksum = work.tile([P, 1], F32, tag="ksum")
                    nc.vector.tensor_reduce(ksum[:], ksq[:], mybir.AxisListType.X,
                                            mybir.AluOpType.add)
                    klog = work.tile([P, 1], F32, tag="kst")
                    nc.scalar.activation(klog[:], ksum[:], AF.Ln, bias=epsp_s[:],
                                         scale=float(1.0 / HD))
                    nc.scalar.activation(rk[:, i:i + 1], klog[:], AF.Exp,
                                         scale=-0.5)
                    ptv = scp.tile([P, 512], F32, tag="sc", name=f"ptrv{ci}")
                    nc.tensor.transpose(ptv[:, :P], vraw[:, j * P:(j + 1) * P], ident_s[:])
                    nc.vector.tensor_copy(vtm[:, i, :], ptv[:, :P])

            # ---- heads (software-pipelined: C1(h+1) stages overlap C2(h)) ----
            ctx = big.tile([P, NH, S], BF)
            qhat_t = {}
            st_qraw = {}
            st_rrec = {}

            def c1a(h):
                # q projection + squared sums; prow matmuls last so the DVE
                # square chain is covered by the second chunk's projection
                qw_h = wstr.tile([P, KT, P], BF, tag="qwh", name=f"qw{h}")
                nc.scalar.dma_start(qw_h[:], qwt_r[:, :, h * P:(h + 1) * P])
                qhat_t[h] = work.tile([P, S], BF, tag="qhat", name=f"qhat{h}")
                q2s = []
                for ci, (c0, cw) in enumerate(CHUNKS):
                    psq = accp.tile([P, 512], F32, tag="acc", name=f"psq{h}")
                    for k in range(KT):
                        nc.tensor.matmul(psq[:, :cw], qw_h[:, k, :], xk(k)[:, c0:c0 + cw],
                                         start=(k == 0), stop=(k == KT - 1))
                    qraw = work.tile([P, 512], BF, tag=f"qraw{ci}", name=f"qraw{h}")
                    nc.vector.tensor_copy(qraw[:, :cw], psq[:, :cw])
                    q2 = work.tile([P, 512], BF, tag="q2", name=f"q2{h}")
                    nc.vector.tensor_tensor(q2[:, :cw], qraw[:, :cw], qraw[:, :cw],
                                            mybir.AluOpType.mult)
                    st_qraw[(h, ci)] = qraw
                    q2s.append(q2)
                for ci, (c0, cw) in enumerate(CHUNKS):
                    prow = rowp.tile([1, 512], F32, tag="row", name=f"prow{h}")
                    nc.tensor.matmul(prow[:, :cw], ones_kb_s[:], q2s[ci][:, :cw],
                                     start=True, stop=True)
                    st_rrec[(h, ci)] = prow

            def c1b(h):
                # rsqrt rows (Ln+Exp keeps the single act table resident)
                for ci, (c0, cw) in enumerate(CHUNKS):
                    prow = st_rrec[(h, ci)]
                    rrow = rows.tile([1, 512], F32, tag="rowa", name=f"rrow{h}")
                    nc.scalar.activation(rrow[:, :cw], prow[:, :cw], AF.Ln,
                                         bias=eps_s[:1, :])
                    rrec = rows.tile([1, 512], F32R, tag="rowb", name=f"rrec{h}")
                    nc.scalar.activation(rrec[:, :cw], rrow[:, :cw], AF.Exp,
                                         scale=-0.5)
                    st_rrec[(h, ci)] = rrec

            def c1c(h):
                # broadcast rows via a tiny PE matmul; consumer reads the PSUM
                qhat = qhat_t[h]
                for ci, (c0, cw) in enumerate(CHUNKS):
                    rrec = st_rrec.pop((h, ci))
                    qraw = st_qraw.pop((h, ci))
                    pbc = bcp.tile([P, 512], F32, tag="bc", name=f"pbc{h}")
                    nc.tensor.matmul(pbc[:, :cw], ones_m_s[:], rrec[:1, :cw],
                                     start=True, stop=True)
                    nc.vector.scalar_tensor_tensor(qhat[:, c0:c0 + cw], qraw[:, :cw],
                                                   qn_s[:], pbc[:, :cw],
                                                   mybir.AluOpType.mult,
                                                   mybir.AluOpType.mult)

            def c2(h, mid_hooks=()):
                qhat = qhat_t.pop(h)
                for ci, (c0, cw) in enumerate(CHUNKS):
                    nvalid = 4 if ci == 0 else 8
                    pctx = accp.tile([P, 512], F32, tag="acc", name=f"pctx{h}")
                    pden = rowp.tile([1, 512], F32, tag="row", name=f"pden{h}")
                    prev = None
                    for ii in range(nvalid):
                        pss = scp.tile([P, 512], F32, tag="sc", name=f"pss{h}")
                        nc.tensor.matmul(pss[:, :cw], kht[:, ii * P:(ii + 1) * P],
                                         qhat[:, c0:c0 + cw], start=True, stop=True)
                        es = work.tile([P, 512], BF, tag="es", bufs=3, name=f"es{h}")
                        mp = ii - 4 * ci
                        lo = mp * P if mp >= 0 else 0
                        # exp only the causally-valid columns; zero the rest
                        nc.scalar.activation(es[:, lo:cw], pss[:, lo:cw], AF.Exp,
                                             scale=rk[:, ii:ii + 1])
                        if mp >= 0:
                            if lo > 0:
                                nc.vector.memset(es[:, :lo], 0.0)
                            nc.vector.tensor_tensor(es[:, lo:lo + P], es[:, lo:lo + P],
                                                    tri_s[:],
                                                    mybir.AluOpType.mult)
                        if ii == 0 and ci < len(mid_hooks):
                            mid_hooks[ci]()
                        if prev is not None:
                            pi, pes = prev
                            nc.tensor.matmul(pctx[:, :cw], vtm[:, pi, :], pes[:, :cw],
                                             start=(pi == 0), stop=False)
                            nc.tensor.matmul(pden[:, :cw], ones_kb_s[:], pes[:, :cw],
                                             start=(pi == 0), stop=False)
                        prev = (ii, es)
                    pi, pes = prev
                    nc.tensor.matmul(pctx[:, :cw], vtm[:, pi, :], pes[:, :cw],
                                     start=(pi == 0), stop=True)
                    nc.tensor.matmul(pden[:, :cw], ones_kb_s[:], pes[:, :cw],
                                     start=(pi == 0), stop=True)
                    # express-release pctx into SBUF so the PSUM bank frees on a
                    # flat-latency copy, then normalize from the broadcast PSUM
                    ctxr = work.tile([P, 512], BF, tag="ctxr", name=f"ctxr{h}")
                    nc.vector.tensor_copy(ctxr[:, :cw], pctx[:, :cw])
                    lnd = rows.tile([1, 512], F32, tag="rowa", name=f"lnd{h}")
                    nc.scalar.activation(lnd[:, :cw], pden[:, :cw], AF.Ln)
                    rden = rows.tile([1, 512], F32R, tag="rowd", name=f"rden{h}")
                    nc.scalar.activation(rden[:, :cw], lnd[:, :cw], AF.Exp,
                                         scale=-1.0)
                    pbcd = bcp.tile([P, 512], F32, tag="bc", name=f"pbcd{h}")
                    nc.tensor.matmul(pbcd[:, :cw], ones_m_s[:], rden[:1, :cw],
                                     start=True, stop=True)
                    nc.vector.tensor_tensor(ctx[:, h, c0:c0 + cw], ctxr[:, :cw],
                                            pbcd[:, :cw], mybir.AluOpType.mult)

            c1a(0)
            c1b(0)
            c1c(0)
            for h in range(NH):
                if h + 1 < NH:
                    c1a(h + 1)
                    c2(h, mid_hooks=(lambda: c1b(h + 1), lambda: c1c(h + 1)))
                else:
                    c2(h)

            # ---- o_proj (partial); stores alternate across both DGE queues ----
            for md in range(KT):
                ow_md = wstr.tile([P, NH, P], BF, tag="owmd")
                nc.sync.dma_start(ow_md[:], owt_r[:, :, md * P:(md + 1) * P])
                for ci, (c0, cw) in enumerate(CHUNKS):
                    pso = accp.tile([P, 512], F32, tag="acc")
                    for h2 in range(NH):
                        nc.tensor.matmul(pso[:, :cw], ow_md[:, h2, :], ctx[:, h2, c0:c0 + cw],
                                         start=(h2 == 0), stop=(h2 == NH - 1))
                    osb = work.tile([P, 512], F32, tag="osb")
                    nc.vector.tensor_copy(osb[:, :cw], pso[:, :cw])
                    eng = nc.scalar if (md * 2 + ci) % 2 == 0 else nc.sync
                    eng.dma_start(part_r[:, md, c0:c0 + cw], osb[:, :cw])

    nc.compile()
    return nc


def _build_moe(segs, CT):
    """segs: list of (offset, cap) per slot (same layout on all cores)."""
    import concourse.tile as tile
    from concourse import bacc, mybir

    F32 = mybir.dt.float32
    BF = mybir.dt.bfloat16
    F8 = mybir.dt.float8e4
    AF = mybir.ActivationFunctionType
    NS = len(segs)
    GU_DESCALE = 1.0 / (W_SCALE * A_SCALE)          # psg -> real gate/up values
    H_FOLD = H_SCALE / (W_SCALE * A_SCALE)          # psu * sg -> hidden * H_SCALE
    DN_DESCALE = 1.0 / (W_SCALE * H_SCALE)          # psd -> real down output

    nc = bacc.Bacc("TRN2", target_bir_lowering=False, debug=False, num_devices=8)
    xgt = nc.dram_tensor("xgt", [D, CT], F8, kind="ExternalInput").ap()
    gwt = nc.dram_tensor("gwt", [NS, D, MI], F8, kind="ExternalInput").ap()
    uwt = nc.dram_tensor("uwt", [NS, D, MI], F8, kind="ExternalInput").ap()
    dnt = nc.dram_tensor("dnt", [NS, MI, D], F8, kind="ExternalInput").ap()
    mout = nc.dram_tensor("mout", [D, CT], BF, kind="ExternalOutput").ap()

    xgt_r = xgt.rearrange("(o p) c -> p o c", p=P)
    mout_r = mout.rearrange("(o p) c -> p o c", p=P)

    with tile.TileContext(nc) as tc:
        with (
            tc.tile_pool(name="xp", bufs=2) as xp,
            tc.tile_pool(name="dnp", bufs=2) as dnp,
            tc.tile_pool(name="wp", bufs=8) as wp,
            tc.tile_pool(name="hp", bufs=2) as hp,
            tc.tile_pool(name="op", bufs=3) as op_,
            tc.tile_pool(name="gps", bufs=1, space="PSUM") as gps,
            tc.tile_pool(name="dps", bufs=2, space="PSUM") as dps,
        ):
            DR = mybir.MatmulPerfMode.DoubleRow

            def emit_down_md(pend, md):
                # one down-proj output tile of the PREVIOUS slot
                dn_p, h_p, off_p, cs_p = pend
                psd = dps.tile([P, 512], F32, tag="d", name=f"psd{md}")
                for jp in range(GM // 2):
                    nc.tensor.matmul(psd[:, :cs_p],
                                     dn_p[:, 2 * jp:2 * jp + 2, md * P:(md + 1) * P],
                                     h_p[:, 2 * jp:2 * jp + 2, :],
                                     start=(jp == 0), stop=(jp == GM // 2 - 1),
                                     perf_mode=DR)
                ob = op_.tile([P, 512], BF, tag="ob", name=f"ob{md}")
                nc.vector.tensor_scalar_mul(ob[:, :cs_p], psd[:, :cs_p],
                                            float(DN_DESCALE))
                nc.scalar.dma_start(mout_r[:, md, off_p:off_p + cs_p], ob[:, :cs_p])

            # warm the Silu act table while the first DMAs are in flight
            warm = hp.tile([P, 1], F32, tag="warm", bufs=1)
            nc.vector.memset(warm[:], 0.0)
            nc.scalar.activation(warm[:], warm[:], AF.Silu)

            # prefetch each slot's tokens one slot ahead on the weight queue
            xg_tiles = {}

            def fetch_xg(s):
                off, cs = segs[s]
                xg_tiles[s] = xp.tile([P, KT, cs], F8, tag="xg", name=f"xg{s}")
                nc.sync.dma_start(xg_tiles[s][:], xgt_r[:, :, off:off + cs])

            fetch_xg(0)
            pend = None
            KP = KT // 2
            MH = GM // 2
            for s, (off, cs) in enumerate(segs):
                xg_s = xg_tiles.pop(s)

                gw_r = gwt[s].rearrange("(o p) i -> p o i", p=P)
                uw_r = uwt[s].rearrange("(o p) i -> p o i", p=P)

                # MI is processed in two halves so gate/up use disjoint PSUM
                # banks: the up pass of half 0 overlaps the gate pass of half 1
                # and the silu never drains the PE.  The previous slot's
                # down-proj tiles interleave into both gate passes.
                gw_t, uw_t = [], []
                hvals = hp.tile([P, GM, cs], F8, tag="h")
                dn_s = None
                for mh in range(2):
                    mlo = mh * MH
                    psg = [gps.tile([P, 512], F32, tag=f"ga{m}",
                                    name=f"psg{mh}_{m}") for m in range(MH)]
                    for kp in range(KP):
                        if mh == 0:
                            gw_k = wp.tile([P, 2, MI], F8, tag="gw", bufs=KP,
                                           name=f"gw{kp}")
                            nc.sync.dma_start(gw_k[:], gw_r[:, 2 * kp:2 * kp + 2, :])
                            uw_k = wp.tile([P, 2, MI], F8, tag="uw", bufs=KP,
                                           name=f"uw{kp}")
                            nc.sync.dma_start(uw_k[:], uw_r[:, 2 * kp:2 * kp + 2, :])
                            gw_t.append(gw_k)
                            uw_t.append(uw_k)
                        for m in range(MH):
                            nc.tensor.matmul(psg[m][:, :cs],
                                             gw_t[kp][:, :, (mlo + m) * P:(mlo + m + 1) * P],
                                             xg_s[:, 2 * kp:2 * kp + 2, :],
                                             start=(kp == 0), stop=(kp == KP - 1),
                                             perf_mode=DR)
                        if pend is not None:
                            emit_down_md(pend, 8 * mh + kp)
                    if mh == 0:
                        if s + 1 < len(segs):
                            fetch_xg(s + 1)
                        dn_s = dnp.tile([P, GM, D], F8, tag="dn")
                        nc.sync.dma_start(dn_s[:], dnt[s].rearrange("(o p) d -> p o d", p=P))
                    sg = hp.tile([P, MH, cs], F32, tag=f"sg{mh}")
                    for m in range(MH):
                        nc.scalar.activation(sg[:, m], psg[m][:, :cs], AF.Silu,
                                             scale=float(GU_DESCALE))
                    psu = [gps.tile([P, 512], F32, tag=f"gb{m}",
                                    name=f"psu{mh}_{m}") for m in range(MH)]
                    for kp in range(KP):
                        for m in range(MH):
                            nc.tensor.matmul(psu[m][:, :cs],
                                             uw_t[kp][:, :, (mlo + m) * P:(mlo + m + 1) * P],
                                             xg_s[:, 2 * kp:2 * kp + 2, :],
                                             start=(kp == 0), stop=(kp == KP - 1),
                                             perf_mode=DR)
                    for m in range(MH):
                        nc.vector.scalar_tensor_tensor(hvals[:, mlo + m], psu[m][:, :cs],
                                                       float(H_FOLD), sg[:, m],
                                                       mybir.AluOpType.mult,
                                                       mybir.AluOpType.mult)
                pend = (dn_s, hvals, off, cs)

            for md in range(KT):
                emit_down_md(pend, md)

    nc.compile()
    return nc


def _refine_logits(tokens, x, in_ln_w, q_w, k_w, v_w, o_w, qn_w, kn_w,
                   post_ln_w, router_w):
    """Exact (fp64) router logits for the given global token ids."""
    out = {}
    nrep = H // HKV
    x64 = x.astype(np.float64)
    qw64 = q_w.astype(np.float64)
    kw64 = k_w.astype(np.float64)
    vw64 = v_w.astype(np.float64)
    ow64 = o_w.astype(np.float64)
    rw64 = router_w.astype(np.float64)
    for b in sorted({int(t) // S for t in tokens}):
        xb = x64[b]
        xn = xb / np.sqrt((xb ** 2).mean(-1, keepdims=True) + EPS) * in_ln_w
        k = (xn @ kw64.T).reshape(S, HKV, HD)
        k = k / np.sqrt((k ** 2).mean(-1, keepdims=True) + EPS) * kn_w
        v = (xn @ vw64.T).reshape(S, HKV, HD)
        for t in [int(t) for t in tokens if int(t) // S == b]:
            p = t % S
            q = (xn[p] @ qw64.T).reshape(H, HD)
            q = q / np.sqrt((q ** 2).mean(-1, keepdims=True) + EPS) * qn_w
            ctx = np.empty((H, HD))
            for h in range(H):
                g = h // nrep
                sc = (k[:p + 1, g] @ q[h]) * (HD ** -0.5)
                eo = np.exp(sc - sc.max())
                ctx[h] = (eo / eo.sum()) @ v[:p + 1, g]
            at = ctx.reshape(-1) @ ow64.T
            h1t = xb[p] + at
            xmt = h1t / np.sqrt((h1t ** 2).mean() + EPS) * post_ln_w
            out[t] = xmt @ rw64.T
    return out


def _run(nc, in_maps, trace):
    from concourse.bass_utils import run_bass_kernel_spmd
    res = run_bass_kernel_spmd(nc, in_maps, core_ids=list(range(8)), trace=trace)
    if trace:
        LAST_EXEC_NS.append(res.exec_time_ns)
        LAST_RESULTS.append(res)
    return res.results


def kernel(x, in_ln_w, q_w, k_w, v_w, o_w, qn_w, kn_w, post_ln_w,
           router_w, gate_up_w, down_w):
    trace = os.environ.get("BASS_KERNEL_TRACE", "0") == "1"
    LAST_EXEC_NS.clear()
    LAST_RESULTS.clear()
    _patch_act_tables()

    x = np.asarray(x, np.float32)
    in_ln_w = np.asarray(in_ln_w, np.float32)
    q_w = np.asarray(q_w, np.float32)
    k_w = np.asarray(k_w, np.float32)
    v_w = np.asarray(v_w, np.float32)
    o_w = np.asarray(o_w, np.float32)
    qn_w = np.asarray(qn_w, np.float32)
    kn_w = np.asarray(kn_w, np.float32)
    post_ln_w = np.asarray(post_ln_w, np.float32)
    router_w = np.asarray(router_w, np.float32)
    gate_up_w = np.asarray(gate_up_w, np.float32)
    down_w = np.asarray(down_w, np.float32)

    # ---------- host prep: pre-normed input, transposed weight shards ----------
    xT = [np.ascontiguousarray(x[b].T) for b in range(B)]          # [D, S]
    rms = 1.0 / np.sqrt((x.astype(np.float64) ** 2).mean(-1) + EPS)  # [B, S]
    xntT = [np.ascontiguousarray(in_ln_w[:, None] * xT[b] * rms[b][None, :].astype(np.float32)).astype(BF16)
            for b in range(B)]

    tri = np.triu(np.ones((P, P), np.float32)).astype(BF16)  # [t', t] valid t>=t'
    ident = np.eye(P, dtype=np.float32)
    ones_kb = np.ones((P, 1), np.float32).astype(BF16)
    ones_m = np.ones((1, P), np.float32)
    qn_col = np.ascontiguousarray(qn_w.reshape(P, 1))
    kn_col = np.ascontiguousarray(kn_w.reshape(P, 1))

    attn_nc = _build_attn()
    in_maps1 = []
    for c in range(8):
        b, g = c // HKV, c % HKV
        qslice = np.ascontiguousarray(q_w[g * NH * HD:(g + 1) * NH * HD].T).astype(BF16)
        kslice = np.ascontiguousarray(k_w[g * HD:(g + 1) * HD].T).astype(BF16)
        vslice = np.ascontiguousarray(v_w[g * HD:(g + 1) * HD].T).astype(BF16)
        oslice = np.ascontiguousarray(o_w[:, g * NH * HD:(g + 1) * NH * HD].T).astype(BF16)
        in_maps1.append({
            "xnt": xntT[b], "qwt": qslice, "kwt": kslice, "vwt": vslice,
            "owt": oslice, "qn": qn_col, "kn": kn_col,
            "ones_kb": ones_kb, "ones_m": ones_m,
            "tri": tri, "ident": ident,
        })
    res1 = _run(attn_nc, in_maps1, trace)

    # ---------- residual + post-norm + routing (host) ----------
    attnT = [res1[4 * b + 0]["part"] + res1[4 * b + 1]["part"]
             + res1[4 * b + 2]["part"] + res1[4 * b + 3]["part"] for b in range(B)]
    if os.environ.get("BASS_KERNEL_DEBUG", "0") == "1":
        np.save("/root/problem/dbg_attnT.npy", np.stack(attnT))
        np.save("/root/problem/dbg_parts.npy",
                np.stack([res1[c]["part"] for c in range(8)]))
    h1T = np.concatenate([xT[b] + attnT[b] for b in range(B)], axis=1)  # [D, T]
    mrms = 1.0 / np.sqrt((h1T.astype(np.float64) ** 2).mean(0) + EPS)   # [T]
    xmT = (post_ln_w[:, None] * h1T * mrms[None, :].astype(np.float32)).astype(np.float32)

    logits = (xmT.T @ router_w.T).astype(np.float32)                    # [T, E]
    lmax = logits.max(-1, keepdims=True)
    ex = np.exp(logits - lmax)
    probs = ex / ex.sum(-1, keepdims=True)
    order = np.argsort(-probs, axis=-1, kind="stable")
    idx = order[:, :TOPK]                                               # [T, 8]
    vals = np.take_along_axis(probs, idx, axis=-1)
    vals = vals / vals.sum(-1, keepdims=True)

    # Top-8 selections whose prob gap is within our attention error bound are
    # ambiguous: recompute those tokens' logits exactly (fp64) on host so the
    # expert choice matches the fp32 reference.
    srt = np.sort(probs, axis=-1)[:, ::-1]
    amb = np.where(srt[:, TOPK - 1] - srt[:, TOPK] < 3e-4)[0]
    if os.environ.get("BASS_KERNEL_DEBUG", "0") == "1":
        print(f"[kernel] ambiguous tokens: {len(amb)}")
    if len(amb):
        refined = _refine_logits(amb, x, in_ln_w, q_w, k_w, v_w, o_w,
                                 qn_w, kn_w, post_ln_w, router_w)
        for t, lg in refined.items():
            eo = np.exp(lg - lg.max())
            pb = eo / eo.sum()
            o8 = np.argsort(-pb, kind="stable")[:TOPK]
            idx[t] = o8
            v8 = pb[o8]
            vals[t] = (v8 / v8.sum()).astype(np.float32)

    # token lists per expert
    tok_ids = [None] * E
    tok_w = [None] * E
    flat_tok = np.repeat(np.arange(T), TOPK)
    flat_e = idx.ravel()
    flat_w = vals.ravel()
    ords = np.argsort(flat_e, kind="stable")
    bounds = np.searchsorted(flat_e[ords], np.arange(E + 1))
    for e in range(E):
        sel = ords[bounds[e]:bounds[e + 1]]
        tok_ids[e] = flat_tok[sel]
        tok_w[e] = flat_w[sel].astype(np.float32)
    counts = np.array([len(t) for t in tok_ids])

    # balanced assignment: rank-grouped — slot s of core c gets expert ranked 8s+c
    rank = np.argsort(-counts, kind="stable")
    assign = [[int(rank[8 * s + c]) for s in range(8)] for c in range(8)]
    caps = []
    for s in range(8):
        cap = int(max(counts[rank[8 * s + c]] for c in range(8)))
        # multiple of 16 so fp8 DoubleRow APs meet the 16B step alignment
        caps.append(max(16, (cap + 15) // 16 * 16))
    offs = np.concatenate([[0], np.cumsum(caps)]).astype(int)
    CT = int(offs[-1])
    segs = [(int(offs[s]), int(caps[s])) for s in range(8)]

    xm_f8 = np.clip(xmT * A_SCALE, -240.0, 240.0).astype(F8E4)
    moe_nc = _build_moe(segs, CT)
    in_maps2 = []
    for c in range(8):
        xg = np.zeros((D, CT), F8E4)
        gw = np.empty((8, D, MI), F8E4)
        uw = np.empty((8, D, MI), F8E4)
        dn = np.empty((8, MI, D), F8E4)
        for s in range(8):
            e = assign[c][s]
            ids = tok_ids[e]
            xg[:, offs[s]:offs[s] + len(ids)] = xm_f8[:, ids]
            gw[s] = np.clip(gate_up_w[e, :MI].T * W_SCALE, -240, 240).astype(F8E4)
            uw[s] = np.clip(gate_up_w[e, MI:].T * W_SCALE, -240, 240).astype(F8E4)
            dn[s] = np.clip(down_w[e].T * W_SCALE, -240, 240).astype(F8E4)
        in_maps2.append({"xgt": xg, "gwt": gw, "uwt": uw, "dnt": dn})
    res2 = _run(moe_nc, in_maps2, trace)

    # ---------- scatter-add + final residual (host) ----------
    moT = np.zeros((D, T), np.float32)
    for c in range(8):
        mo = res2[c]["mout"].astype(np.float32)
        for s in range(8):
            e = assign[c][s]
            ids = tok_ids[e]
            if len(ids):
                moT[:, ids] += tok_w[e][None, :] * mo[:, offs[s]:offs[s] + len(ids)]

    if os.environ.get("BASS_KERNEL_DEBUG", "0") == "1":
        np.save("/root/problem/dbg_xmT.npy", xmT)
        np.save("/root/problem/dbg_idx.npy", idx)
        np.save("/root/problem/dbg_vals.npy", vals)
        np.save("/root/problem/dbg_moT.npy", moT)

    outT = h1T + moT
    return np.ascontiguousarray(outT.T).reshape(B, S, D).astype(np.float32)
